# revision 59
# baseline (speedup 1.0000x reference)
"""Trainium2 Bass kernel v6: packed-complex Wiener deconvolution.

v6 over v5: even/odd channel pairs packed as complex rows (halves the
spectrum-multiply, M3 matmuls, T2 transposes, and evac traffic); filter
spectrum Hermitian-ized (g~ = (g[k]+conj(g[-k]))/2) via a P,Q dual-DFT with
sign-baked +/- M2 weight sets (no conj-flip indexing); |h|^2 pair-sum and the
+s regularizer folded into PE matmuls; bias enters through the M3 DC bin as a
rank-1 PE seed; M4 uses 3 cL components (Lr, -Li, +Li); the whole H->G chain
is quarter-pipelined across PE/ACT/DVE; Pool handles the (SBUF-only) x-twiddle
and part of the spectrum multiplies.
"""
import sys

sys.path.insert(0, "/opt/trn_rl_repo")

import numpy as np


def _get_cc():
    import concourse.bacc as bacc
    import concourse.mybir as mybir
    import concourse.tile as tile
    return bacc, mybir, tile


class Cfg:
    def __init__(self, T=8192, N2=128, N1=64, BL=2, C=8, FIL=16):
        assert N1 * N2 == T
        self.T, self.N2, self.N1, self.BL, self.C, self.FIL = T, N2, N1, BL, C, FIL
        self.CP = C // 2
        self.FC = FIL * C


FULL = Cfg()


def host_consts(cfg):
    T, N1, N2, FIL, CP = cfg.T, cfg.N1, cfg.N2, cfg.FIL, cfg.CP
    f32 = np.float32
    n2a, n1a, k2a, k1a = (np.arange(N2), np.arange(N1), np.arange(N2), np.arange(N1))
    cs = {}
    F2 = np.exp(-2j * np.pi * np.outer(n2a, k2a) / N2)          # [n2,k2]
    cs["blob_r"] = np.concatenate(
        [F2.real, F2.imag, -F2.imag], axis=1).astype(f32)
    Tw = np.exp(-2j * np.pi * np.outer(k2a, n1a) / T)           # [k2,n1]
    tw3 = np.concatenate([Tw.real, Tw.imag, -Tw.imag], axis=1).astype(f32)
    cs["blob_twb"] = tw3                                        # ->bf16 twiddles
    F1 = np.exp(-2j * np.pi * np.outer(n1a, k1a) / N1)          # [n1,k1]
    Wstd = np.vstack([np.hstack([F1.real, F1.imag]),
                      np.hstack([-F1.imag, F1.real])]).astype(f32)  # [n1s,k1s]
    Wre, Wim = Wstd[:, :N1], Wstd[:, N1:]
    # set+ out rows: [h+r; -h+i] = [Pr - Qi; -(Pi + Qr)]
    Wp_p = np.hstack([Wre, -Wim])
    Wq_p = np.hstack([-Wim, -Wre])
    # set- out rows: [h-r; h-i] = [Pr + Qi; -Pi + Qr]
    Wp_m = np.hstack([Wre, -Wim])
    Wq_m = np.hstack([Wim, Wre])
    cs["blob_m2"] = np.concatenate([Wstd, Wp_p, Wq_p, Wp_m, Wq_m], axis=1)
    F1b = np.exp(+2j * np.pi * np.outer(k1a, n1a) / N1)         # [k1,j]
    Fbr, Fbi = F1b.real, F1b.imag
    M3A = np.hstack([np.vstack([Fbr, -Fbi]), np.vstack([Fbi, Fbr])]) * 0.5
    M3B = np.hstack([np.vstack([-Fbi, -Fbr]), np.vstack([Fbr, -Fbi])]) * 0.5
    cs["blob_m3"] = np.concatenate([M3A, M3B], axis=1).astype(f32)
    I64 = np.eye(N1, dtype=f32)
    Spair = np.vstack([I64, I64])
    cs["blob_sel"] = np.hstack([Spair, Spair]).astype(f32)      # [k1s, 128]
    ia = np.arange(N2)
    L = np.exp(2j * np.pi * (np.outer(k2a, ia * N1)[:, None, :]
                             + k2a[:, None, None] * n1a[None, :, None]) / T) / T
    cs["cLAB"] = np.concatenate(
        [L.real.reshape(N2, N1 * N2), -L.imag.reshape(N2, N1 * N2),
         L.imag.reshape(N2, N1 * N2)],
        axis=1).astype(f32)                                     # [k2,(n1p,i)x3]
    return cs


def build_nc(cfg):
    bacc, mybir, tile = _get_cc()
    F32, F32R, BF16 = mybir.dt.float32, mybir.dt.float32r, mybir.dt.bfloat16
    AL = mybir.AluOpType
    T, N2, N1, BL, C, FIL, CP = (cfg.T, cfg.N2, cfg.N1, cfg.BL, cfg.C,
                                 cfg.FIL, cfg.CP)
    FC = cfg.FC
    N1s = 2 * N1                  # 128
    KF = FIL * N2                 # 2048
    HN = FIL * N1                 # 1024
    XN = BL * N1 * C              # 1024 (dram x layout, c innermost)
    CK = CP * N2                  # 512
    FH = FIL // 2                 # 8 filters per psum half-batch
    NF4 = FIL // 4                # 4 filters per G quarter
    QW = KF // 4                  # 512
    MCH = 512

    nc = bacc.Bacc("TRN2", debug=False)

    xs_d = nc.dram_tensor("xs", [N2, XN], F32R, kind="ExternalInput")
    wr_d = nc.dram_tensor("wr", [N2, HN], F32R, kind="ExternalInput")
    wi_d = nc.dram_tensor("wi", [N2, HN], F32R, kind="ExternalInput")
    blob_r_d = nc.dram_tensor("blob_r", [N2, 3 * N2], F32R, kind="ExternalInput")
    blob_twb_d = nc.dram_tensor("blob_twb", [N2, 3 * N1], BF16, kind="ExternalInput")
    blob_m2_d = nc.dram_tensor("blob_m2", [N1s, 5 * N1s], BF16, kind="ExternalInput")
    blob_m3_d = nc.dram_tensor("blob_m3", [N1s, 2 * N1s], BF16, kind="ExternalInput")
    blob_sel_d = nc.dram_tensor("blob_sel", [N1s, N1s], BF16, kind="ExternalInput")
    seeds_d = nc.dram_tensor("seeds", [1, KF + 3 * N1s + 2 * FIL * CP], BF16,
                             kind="ExternalInput")
    cLAB_d = nc.dram_tensor("cLAB", [N2, 3 * N1 * N2], BF16, kind="ExternalInput")
    out_d = nc.dram_tensor("out", [BL, T, FC], BF16, kind="ExternalOutput")

    with tile.TileContext(nc) as tc:
        from contextlib import ExitStack
        with tc.tile_pool(name="consts", bufs=1) as cpool, \
             tc.tile_pool(name="pers", bufs=1) as pers:
            # ---------- loads (SP queue, in order) ----------
            def load(name, shape, dt, dram):
                t = cpool.tile(shape, dt, tag=name, name=name)
                nc.sync.dma_start(out=t, in_=dram.ap())
                return t

            blob_r = load("blob_r", [N2, 3 * N2], F32R, blob_r_d)
            wtr = load("wtr", [N2, HN], F32R, wr_d)
            wti = load("wti", [N2, HN], F32R, wi_d)
            blob_twb = load("blob_twb", [N2, 3 * N1], BF16, blob_twb_d)
            blob_m2 = load("blob_m2", [N1s, 5 * N1s], BF16, blob_m2_d)
            blob_m3 = load("blob_m3", [N1s, 2 * N1s], BF16, blob_m3_d)
            blob_sel = load("blob_sel", [N1s, N1s], BF16, blob_sel_d)
            seeds = load("seeds", [1, KF + 3 * N1s + 2 * FIL * CP], BF16, seeds_d)
            xt = load("xt", [N2, XN], F32R, xs_d)
            cLAB = cpool.tile([N2, 3 * N1 * N2], BF16, tag="cLAB")
            QL = N1 * N2 // 4                                   # 2048

            F2r = blob_r[:, 0:N2]
            F2i = blob_r[:, N2:2 * N2]
            F2in = blob_r[:, 2 * N2:3 * N2]
            twrb = blob_twb[:, 0:N1]
            twib = blob_twb[:, N1:2 * N1]
            twinb = blob_twb[:, 2 * N1:3 * N1]
            cM2x = blob_m2[:, 0:N1s]
            Wp_p = blob_m2[:, N1s:2 * N1s]
            Wq_p = blob_m2[:, 2 * N1s:3 * N1s]
            Wp_m = blob_m2[:, 3 * N1s:4 * N1s]
            Wq_m = blob_m2[:, 4 * N1s:5 * N1s]
            cM3A = blob_m3[:, 0:N1s]
            cM3B = blob_m3[:, N1s:2 * N1s]
            selA = blob_sel[:, 0:N1s]
            srow = seeds[:, 0:KF]
            ones1 = seeds[:, KF:KF + N1s]
            cselRe = seeds[:, KF + N1s:KF + 2 * N1s]
            cselIm = seeds[:, KF + 2 * N1s:KF + 3 * N1s]
            seedRe = seeds[:, KF + 3 * N1s:KF + 3 * N1s + FIL * CP]
            seedIm = seeds[:, KF + 3 * N1s + FIL * CP:]
            cLA = cLAB[:, 0:N1 * N2]
            cLB = cLAB[:, N1 * N2:2 * N1 * N2]
            cLC = cLAB[:, 2 * N1 * N2:3 * N1 * N2]

            # persistent tiles
            Z0A = pers.tile([N1s, BL * CK], BF16, tag="Z0A")     # [k1s,(b,cp,k2)]
            Grep = pers.tile([N1s, FIL * 2 * N2], BF16, tag="Grep")
            Grv = Grep.rearrange("p (f m q) -> p f m q", f=FIL, m=2)
            hpm = pers.tile([N1s, KF], BF16, tag="hpm")          # [h+r; -h+i]
            hmm = pers.tile([N1s, KF], BF16, tag="hmm")          # [h-r; h-i]
            sqp = pers.tile([N1s, KF], BF16, tag="sqp")
            sqm = pers.tile([N1s, KF], BF16, tag="sqm")
            RP = pers.tile([N1s, KF], BF16, tag="RP")            # r+ both halves
            RM = pers.tile([N1s, KF], BF16, tag="RM")            # r- both halves
            G12s = pers.tile([N1s, QW], BF16, tag="G12s")
            SS2p = pers.tile([N1s, QW], BF16, tag="SS2p")
            SS2m = pers.tile([N1s, QW], BF16, tag="SS2m")

            fes = ExitStack()
            fwd = fes.enter_context(tc.tile_pool(name="fwd", bufs=1))
            pxes = ExitStack()
            pAx = pxes.enter_context(tc.tile_pool(name="pAx", bufs=1, space="PSUM"))
            phes = ExitStack()
            pAh = phes.enter_context(tc.tile_pool(name="pAh", bufs=1, space="PSUM"))

            # ---------- M1: H quarter-pipelined, x interleaved ----------
            SH = fwd.tile([N2, 2 * 2 * FIL * N1], BF16, tag="SH")
            SHv = SH.rearrange("p (g m f n) -> p g m f n", g=2, m=2, f=FIL)
            SX = fwd.tile([N2, 2 * BL * CP * N1], BF16, tag="SX")
            SXv = SX.rearrange("p (m b c n) -> p m b c n", m=2, b=BL, c=CP)
            wtrv = wtr.rearrange("p (f n) -> p f n", f=FIL)
            wtiv = wti.rearrange("p (f n) -> p f n", f=FIL)
            Ax = pAx.tile([N2, 2 * BL * CP * N1], F32, tag="Ax")  # [k2,(m,b,cp,n1)]
            Axv = Ax.rearrange("p (m b c n) -> p m b c n", m=2, b=BL, c=CP)
            xtv = xt.rearrange("p (b n c e) -> p b n c e", b=BL, n=N1, c=CP)
            xe = xtv[:, :, :, :, 0].transpose([0, 1, 3, 2])      # [n2,(b,cp,n1)]
            xo = xtv[:, :, :, :, 1].transpose([0, 1, 3, 2])

            def m1h_q(q):
                Ahh = pAh.tile([N2, 2 * 2 * NF4 * N1], F32, tag="Ah", name=f"Ah{q}")
                Av = Ahh.rearrange("p (g m f n) -> p g m f n", g=2, m=2, f=NF4)
                fsl = slice(q * NF4, (q + 1) * NF4)
                nc.tensor.matmul(Av[:, 0, 0], F2r, wtrv[:, fsl], start=True, stop=True)
                nc.tensor.matmul(Av[:, 0, 1], F2i, wtrv[:, fsl], start=True, stop=True)
                nc.tensor.matmul(Av[:, 1, 0], F2r, wtiv[:, fsl], start=True, stop=True)
                nc.tensor.matmul(Av[:, 1, 1], F2i, wtiv[:, fsl], start=True, stop=True)
                nc.scalar.copy(out=SHv[:, 0, :, fsl, :], in_=Av[:, 0])
                nc.scalar.copy(out=SHv[:, 1, :, fsl, :], in_=Av[:, 1])

            m1h_q(0)
            m1h_q(1)
            nc.tensor.matmul(Axv[:, 0], F2r, xe, start=True, stop=False)
            nc.tensor.matmul(Axv[:, 0], F2in, xo, start=False, stop=True)
            nc.tensor.matmul(Axv[:, 1], F2i, xe, start=True, stop=False)
            nc.tensor.matmul(Axv[:, 1], F2r, xo, start=False, stop=True)
            nc.scalar.copy(out=SX, in_=Ax)
            m1h_q(2)
            m1h_q(3)

            # ---------- H twiddle (DVE, bf16 2x), per quarter ----------
            Bh = fwd.tile([N2, 2 * FIL * 2 * N1], BF16, tag="Bh")
            Bhv = Bh.rearrange("p (u g f m n) -> p u g f m n", u=4, g=2, f=NF4, m=2)
            uh = fwd.tile([N2, NF4 * N1], BF16, tag="uh")
            vh = fwd.tile([N2, NF4 * N1], BF16, tag="vh")
            uhv = uh.rearrange("p (f n) -> p f n", f=NF4)
            vhv = vh.rearrange("p (f n) -> p f n", f=NF4)

            def bch(w):
                return w[:, None, :].broadcast_to([N2, NF4, N1])

            BTH = fwd.tile([N1s, 2 * 2 * FH * N2], BF16, tag="BTH")
            BTHg = BTH.rearrange("p (u g f q) -> p u g f q", u=4, g=2, f=NF4)

            uh2 = fwd.tile([N2, NF4 * N1], BF16, tag="uh2")
            vh2 = fwd.tile([N2, NF4 * N1], BF16, tag="vh2")
            uh2v = uh2.rearrange("p (f n) -> p f n", f=NF4)
            vh2v = vh2.rearrange("p (f n) -> p f n", f=NF4)

            def htw_q(u, eng=None, us=None, vs=None):
                eng = eng or nc.vector
                us, vs = us or uhv, vs or vhv
                fsl = slice(u * NF4, (u + 1) * NF4)
                for g in range(2):    # P, Q
                    eng.tensor_tensor(out=us, in0=SHv[:, g, 0, fsl, :],
                                      in1=bch(twrb), op=AL.mult)
                    eng.tensor_tensor(out=vs, in0=SHv[:, g, 1, fsl, :],
                                      in1=bch(twinb), op=AL.mult)
                    eng.tensor_tensor(out=Bhv[:, u, g, :, 0, :], in0=us,
                                      in1=vs, op=AL.mult if False else AL.add)
                    eng.tensor_tensor(out=us, in0=SHv[:, g, 0, fsl, :],
                                      in1=bch(twib), op=AL.mult)
                    eng.tensor_tensor(out=vs, in0=SHv[:, g, 1, fsl, :],
                                      in1=bch(twrb), op=AL.mult)
                    eng.tensor_tensor(out=Bhv[:, u, g, :, 1, :], in0=us,
                                      in1=vs, op=AL.add)

            def t1h_q(u):
                nc.sync.dma_start_transpose(
                    out=BTHg[:, u].rearrange("p g f q -> p (g f) q"),
                    in_=Bhv[:, u].rearrange("p g f m n -> p (g f) (m n)"))

            htw_q(0)
            htw_q(1)
            htw_q(3)

            # ---------- x twiddle (Pool, staged bf16 SBUF), split per b ----------
            Bc = fwd.tile([N2, BL * CP * 2 * N1], BF16, tag="Bc")
            Bcv = Bc.rearrange("p (b c m n) -> p b c m n", b=BL, c=CP, m=2)
            ux = fwd.tile([N2, CP * N1], BF16, tag="ux")
            vx = fwd.tile([N2, CP * N1], BF16, tag="vx")
            uxv = ux.rearrange("p (c n) -> p c n", c=CP)
            vxv = vx.rearrange("p (c n) -> p c n", c=CP)

            def bcx(w):
                return w[:, None, :].broadcast_to([N2, CP, N1])

            def xtw_b(b):
                nc.gpsimd.tensor_tensor(out=uxv, in0=SXv[:, 0, b], in1=bcx(twrb),
                                        op=AL.mult)
                nc.gpsimd.tensor_tensor(out=vxv, in0=SXv[:, 1, b], in1=bcx(twinb),
                                        op=AL.mult)
                nc.gpsimd.tensor_tensor(out=Bcv[:, b, :, 0, :], in0=uxv, in1=vxv,
                                        op=AL.add)
                nc.gpsimd.tensor_tensor(out=uxv, in0=SXv[:, 0, b], in1=bcx(twib),
                                        op=AL.mult)
                nc.gpsimd.tensor_tensor(out=vxv, in0=SXv[:, 1, b], in1=bcx(twrb),
                                        op=AL.mult)
                nc.gpsimd.tensor_tensor(out=Bcv[:, b, :, 1, :], in0=uxv, in1=vxv,
                                        op=AL.add)

            xtw_b(1)
            htw_q(2, eng=nc.gpsimd, us=uh2v, vs=vh2v)
            xtw_b(0)

            # ---------- T1s (SP, ordered by expected readiness) ----------
            BTx = fwd.tile([N1s, BL * CP * N2], BF16, tag="BTx")   # [n1s,(b,cp,k2)]
            BTxv = BTx.rearrange("p (b c q) -> p b c q", b=BL, c=CP)

            def t1x_b(b):
                nc.sync.dma_start_transpose(
                    out=BTxv[:, b].rearrange("p c q -> p c q"),
                    in_=Bcv[:, b].rearrange("p c m n -> p c (m n)"))

            t1h_q(0)
            t1x_b(1)
            t1h_q(1)
            t1h_q(3)
            t1h_q(2)
            t1x_b(0)
            for qc in range(12):
                nc.sync.dma_start(out=cLAB[:, qc * QL:(qc + 1) * QL],
                                  in_=cLAB_d.ap()[:, qc * QL:(qc + 1) * QL])

            # ---------- quarter-pipelined M2h/squares/SS/recip + M2x ----------
            phes.close()
            pxes.close()
            pZes = ExitStack()
            pZ = pZes.enter_context(tc.tile_pool(name="pZ", bufs=1, space="PSUM"))
            pHes = ExitStack()
            pH = pHes.enter_context(tc.tile_pool(name="pH", bufs=3, space="PSUM"))
            pSes = ExitStack()
            pS = pSes.enter_context(tc.tile_pool(name="pS", bufs=2, space="PSUM"))

            def m2h_q(q):
                """Quarter q of both +/- sets -> hpm/hmm + squares."""
                qs = slice(q * QW, (q + 1) * QW)
                for (Wp_, Wq_, dsth, dstsq, nm) in (
                        (Wp_p, Wq_p, hpm, sqp, "p"), (Wp_m, Wq_m, hmm, sqm, "m")):
                    Hq = pH.tile([N1s, QW], F32, tag="Hq", name=f"Hq{nm}{q}")
                    nc.tensor.matmul(Hq, Wp_, BTHg[:, q, 0].rearrange(
                        "p f q -> p (f q)"), start=True, stop=False)
                    nc.tensor.matmul(Hq, Wq_, BTHg[:, q, 1].rearrange(
                        "p f q -> p (f q)"), start=False, stop=True)
                    nc.scalar.square(dstsq[:, qs], Hq)
                    nc.scalar.copy(out=dsth[:, qs], in_=Hq)

            def ss_q(q, stage=False):
                qs = slice(q * QW, (q + 1) * QW)
                for (sqt, rrt, st) in ((sqp, RP, SS2p), (sqm, RM, SS2m)):
                    nm = "p" if sqt is sqp else "m"
                    SSq = pS.tile([N1s, QW], F32, tag="SSq", name=f"SS{nm}{q}")
                    nc.tensor.matmul(SSq, selA, sqt[:, qs], start=True, stop=False)
                    nc.tensor.matmul(SSq, ones1, srow[:, qs], start=False, stop=True)
                    if stage:
                        nc.scalar.copy(out=st, in_=SSq)
                    else:
                        with nc.allow_low_precision(reason="bf16 wiener gain"):
                            nc.vector.reciprocal(out=rrt[:, qs], in_=SSq)

            def recip_q2():
                qs = slice(2 * QW, 3 * QW)
                with nc.allow_low_precision(reason="bf16 wiener gain"):
                    nc.vector.reciprocal(out=RP[:, qs], in_=SS2p)
                    nc.vector.reciprocal(out=RM[:, qs], in_=SS2m)

            def g_quarter(q):
                # G12 rows: [G1(k1); G2(k1)] = hpm*RP + hmm*RM (all aligned)
                qs = slice(q * QW, (q + 1) * QW)
                fq = slice(q * NF4, (q + 1) * NF4)
                nc.vector.tensor_tensor(out=G12s, in0=hpm[:, qs], in1=RP[:, qs],
                                        op=AL.mult)
                hmv = hmm.rearrange("p (f q) -> p f q", f=FIL)[:, fq, :]
                rmv = RM.rearrange("p (f q) -> p f q", f=FIL)[:, fq, :]
                nc.vector.tensor_tensor(out=Grv[:, fq, 0, :], in0=hmv, in1=rmv,
                                        op=AL.mult)
                g12v = G12s.rearrange("p (f q) -> p f q", f=NF4)
                nc.vector.tensor_tensor(out=Grv[:, fq, 0, :],
                                        in0=Grv[:, fq, 0, :], in1=g12v, op=AL.add)
                # rows now [G1; G2] in slot m=0; scatter to (m, halves)
                nc.vector.tensor_copy(out=Grv[:N1, fq, 1, :], in_=Grv[N1:, fq, 0, :])
                nc.vector.tensor_copy(out=Grv[N1:, fq, 1, :], in_=Grv[N1:, fq, 0, :])
                nc.vector.tensor_copy(out=Grv[N1:, fq, 0, :], in_=Grv[:N1, fq, 0, :])

            # software-pipelined emission: PE one stage ahead of evac deps
            Zps = pZ.tile([N1s, BL * CK], F32, tag="Zps")

            def m2x_b(b):
                bsl = slice(b * CK, (b + 1) * CK)
                nc.tensor.matmul(Zps[:, bsl], cM2x, BTx[:, bsl], start=True,
                                 stop=True)
                nc.scalar.copy(out=Z0A[:, bsl], in_=Zps[:, bsl])

            m2h_q(0)
            m2h_q(1)
            ss_q(0)
            m2x_b(1)
            m2h_q(3)
            ss_q(1)
            ss_q(3)
            g_quarter(0)
            m2h_q(2)
            m2x_b(0)
            ss_q(2)
            pSes.close()
            pHes.close()
            pZes.close()
            fes.close()

            # ================= inverse =================
            zvA = Z0A.rearrange("p (b c q) -> p b c q", b=BL, c=CP)
            with tc.tile_pool(name="dt", bufs=1) as dtp, \
                 tc.tile_pool(name="stg", bufs=1) as stp, \
                 tc.tile_pool(name="zt", bufs=4) as ztp, \
                 tc.tile_pool(name="cse", bufs=5) as csp, \
                 tc.tile_pool(name="ddp", bufs=2, space="PSUM") as ddp, \
                 tc.tile_pool(name="yp", bufs=2, space="PSUM") as yps:
                DT0 = dtp.tile([N2, FIL * CP * N1s], BF16, tag="DT0")
                DT1 = dtp.tile([N2, FIL * CP * N1s], BF16, tag="DT1")
                DT = [DT0, DT1]                       # [k2,(f,cp,m,n1')]
                STG0 = stp.tile([N2, N1 * FC], BF16, tag="STG0")
                STG1 = stp.tile([N2, N1 * FC], BF16, tag="STG1")
                STG = [STG0, STG1]                    # [i,(j,f,cp,m)]
                def new_cseq():
                    return csp.tile([N1s, 4 * CK], BF16, tag="cseq", name="cseq")
                srv = seedRe.rearrange("o (f c) -> o f c", f=FIL)
                siv = seedIm.rearrange("o (f c) -> o f c", f=FIL)

                # Pool zt pairs are prefetched one loop-step early
                POOL_ZT = {(1, 2), (1, 5), (0, 2), (0, 5)}

                def zt_mul(b, fp, eng):
                    f0 = 2 * fp
                    zt = ztp.tile([N1s, 2 * 2 * CK], BF16, tag="zt")
                    ztv = zt.rearrange("p (i m c q) -> p i m c q", i=2, m=2, c=CP)
                    g12 = Grv[:, f0:f0 + 2][:, :, :, None, :].broadcast_to(
                        [N1s, 2, 2, CP, N2])
                    zin = zvA[:, b][:, None, None, :, :].broadcast_to(
                        [N1s, 2, 2, CP, N2])
                    eng.tensor_tensor(out=ztv, in0=zin, in1=g12, op=AL.mult)
                    return zt

                PENDING_ZT = {}

                def cmul_m3_pair(b, fp, cseq):
                    """f = 2*fp, 2*fp+1: 8+4 matmuls, one evac (ACT)."""
                    f0 = 2 * fp
                    zt = PENDING_ZT.pop((b, fp), None)
                    if zt is None:
                        zt = zt_mul(b, fp, nc.vector)
                    ztv = zt.rearrange("p (i m c q) -> p i m c q", i=2, m=2, c=CP)
                    DD = ddp.tile([N1s, 2 * CK], F32, tag="DD")
                    for i in range(2):
                        f = f0 + i
                        sl = DD[:, i * CK:(i + 1) * CK]
                        nc.tensor.matmul(sl, cM3A, ztv[:, i, 0].rearrange(
                            "p c q -> p (c q)"), start=True, stop=False)
                        nc.tensor.matmul(sl, cM3B, ztv[:, i, 1].rearrange(
                            "p c q -> p (c q)"), start=False, stop=False)
                        DDv = sl.rearrange("p (c q) -> p c q", c=CP)
                        nc.tensor.matmul(DDv[:, :, 0:1], cselRe,
                                         srv[:, f, :, None], start=False, stop=False)
                        nc.tensor.matmul(DDv[:, :, 0:1], cselIm,
                                         siv[:, f, :, None], start=False, stop=True)
                    dst = cseq[:, (fp % 2) * 2 * CK:(fp % 2 + 1) * 2 * CK]
                    nc.scalar.copy(out=dst, in_=DD)

                def prefetch_pool_zt(b, fp):
                    if (b, fp) in POOL_ZT:
                        PENDING_ZT[(b, fp)] = zt_mul(b, fp, nc.gpsimd)

                def quad_t2(b, qf, cseq):
                    dtv = DT[b].rearrange("p (f c n) -> p (f c) n", f=FIL, c=CP)
                    nc.sync.dma_start_transpose(
                        out=dtv[:, qf * 16:(qf + 1) * 16, :], in_=cseq)

                def pair_t2(b, fp, cseq):
                    dtv = DT[b].rearrange("p (f c n) -> p (f c) n", f=FIL, c=CP)
                    sl = cseq[:, (fp % 2) * 2 * CK:(fp % 2 + 1) * 2 * CK]
                    nc.sync.dma_start_transpose(
                        out=dtv[:, fp * 8:(fp + 1) * 8, :], in_=sl)

                def m4_group8(b, g0, eng="act"):
                    """8 n1p values; ypsum [i,(j8,m,fc64)]; one evac."""
                    dtm = DT[b].rearrange("p (f c m n) -> p n m f c",
                                          f=FIL, c=CP, m=2)
                    ypsum = yps.tile([N2, 8 * 2 * N1], F32, tag="yps")
                    ypv = ypsum.rearrange("p (j m o) -> p j m o", j=8, m=2)
                    for j in range(8):
                        n1p = g0 + j
                        wA = cLA[:, n1p * N2:(n1p + 1) * N2]
                        wB = cLB[:, n1p * N2:(n1p + 1) * N2]
                        wC = cLC[:, n1p * N2:(n1p + 1) * N2]
                        dr = dtm[:, n1p, 0]
                        di = dtm[:, n1p, 1]
                        nc.tensor.matmul(ypv[:, j, 0], wA, dr, start=True, stop=False)
                        nc.tensor.matmul(ypv[:, j, 0], wB, di, start=False, stop=True)
                        nc.tensor.matmul(ypv[:, j, 1], wA, di, start=True, stop=False)
                        nc.tensor.matmul(ypv[:, j, 1], wC, dr, start=False, stop=True)
                    dst = STG[b].rearrange("p (n f c m) -> p n f c m",
                                           n=N1, f=FIL, c=CP)[:, g0:g0 + 8]
                    src = ypv.rearrange("p j m (f c) -> p j f c m", f=FIL)
                    if eng == "act":
                        nc.scalar.copy(out=dst, in_=src)
                    else:
                        nc.vector.tensor_copy(out=dst, in_=src)

                def m4_group8_h(b, g0, fh, eng="act"):
                    """8 n1p values, HALF the filters (fh*8..fh*8+8)."""
                    dtm = DT[b].rearrange("p (f c m n) -> p n m f c",
                                          f=FIL, c=CP, m=2)
                    fsl = slice(fh * FH, (fh + 1) * FH)
                    ypsum = yps.tile([N2, 8 * 2 * N1], F32, tag="yps")
                    ypv = ypsum.rearrange("p (j m o) -> p j m o", j=8, m=2)
                    hw2 = FH * CP
                    for j in range(8):
                        n1p = g0 + j
                        wA = cLA[:, n1p * N2:(n1p + 1) * N2]
                        wB = cLB[:, n1p * N2:(n1p + 1) * N2]
                        wC = cLC[:, n1p * N2:(n1p + 1) * N2]
                        dr = dtm[:, n1p, 0, fsl, :]
                        di = dtm[:, n1p, 1, fsl, :]
                        o0 = ypv[:, j, 0, :hw2]
                        o1 = ypv[:, j, 1, :hw2]
                        nc.tensor.matmul(o0, wA, dr, start=True, stop=False)
                        nc.tensor.matmul(o0, wB, di, start=False, stop=True)
                        nc.tensor.matmul(o1, wA, di, start=True, stop=False)
                        nc.tensor.matmul(o1, wC, dr, start=False, stop=True)
                    dst = STG[b].rearrange("p (n f c m) -> p n f c m",
                                           n=N1, f=FIL, c=CP)[:, g0:g0 + 8, fsl]
                    src_ = ypv[:, :, :, :hw2].rearrange(
                        "p j m (f c) -> p j f c m", f=FH)
                    if eng == "act":
                        nc.scalar.copy(out=dst, in_=src_)
                    else:
                        nc.vector.tensor_copy(out=dst, in_=src_)

                def out_chunk(b, g0, gn=16):
                    nc.scalar.dma_start(
                        out=out_d.ap()[b].rearrange(
                            "(q n) fc -> q (n fc)", n=N1)[:, g0 * FC:(g0 + gn) * FC],
                        in_=STG[b][:, g0 * FC:(g0 + gn) * FC])

                EV = ["dve", "act"]
                # ----- loop 1: b=1 M3 (8 pair-steps) -----
                cseq = new_cseq()
                for fp in range(FIL // 2):
                    if fp in (1, 3, 5):
                        g_quarter((fp + 1) // 2)
                    prefetch_pool_zt(1, fp + 1)
                    cmul_m3_pair(1, fp, cseq)
                    if fp % 2 == 1:
                        quad_t2(1, fp // 2, cseq)
                        cseq = new_cseq()
                prefetch_pool_zt(0, 0)
                # ----- loop 2: b=0 M3 + all 8 b=1 M4 super-groups -----
                for fp in range(FIL // 2):
                    if fp < 6:
                        m4_group8(1, fp * 8, EV[fp % 2])
                        if fp % 2 == 1 and fp >= 3:
                            out_chunk(1, (fp - 3) * 8)
                        prefetch_pool_zt(0, fp + 1)
                        cmul_m3_pair(0, fp, cseq)
                    else:
                        prefetch_pool_zt(0, fp + 1)
                        cmul_m3_pair(0, fp, cseq)
                        m4_group8(1, fp * 8, EV[fp % 2])
                    if fp >= 6:
                        pair_t2(0, fp, cseq)
                        if fp % 2 == 1:
                            cseq = new_cseq()
                    elif fp % 2 == 1:
                        quad_t2(0, fp // 2, cseq)
                        cseq = new_cseq()
                out_chunk(1, 32)
                out_chunk(1, 48)
                # ----- b=0 M4 tail -----
                for g in range(8):
                    m4_group8(0, g * 8, EV[g % 2])
                    if g % 2 == 1 and g < 7:
                        out_chunk(0, g * 8 - 8)
                out_chunk(0, 48, gn=12)
                out_chunk(0, 60, gn=4)

    nc.compile()
    return nc


def host_inputs(cfg, x_sh, w_real, w_imag, s, b):
    """Per-core in_map (numpy) for one core's batch shard. Layout-only on
    inputs; constants precomputed."""
    import ml_dtypes
    cs = host_consts(cfg)
    T, N1, N2, FIL, C, CP, BL = (cfg.T, cfg.N1, cfg.N2, cfg.FIL, cfg.C,
                                 cfg.CP, cfg.BL)
    FC, KF, N1s = cfg.FC, FIL * N2, 2 * N1
    f32, bf16 = np.float32, ml_dtypes.bfloat16
    x_sh = np.asarray(x_sh, f32)
    xs = np.ascontiguousarray(
        x_sh.reshape(BL, N2, N1, C).transpose(1, 0, 2, 3)).reshape(N2, BL * N1 * C)
    wr = np.ascontiguousarray(
        np.asarray(w_real, f32).reshape(FIL, N2, N1).transpose(1, 0, 2)
    ).reshape(N2, FIL * N1)
    wi = np.ascontiguousarray(
        np.asarray(w_imag, f32).reshape(FIL, N2, N1).transpose(1, 0, 2)
    ).reshape(N2, FIL * N1)
    sv = np.asarray(s, f32).reshape(FIL)
    srow = np.repeat(sv, N2)[None, :]                       # [1,(f,k2)]
    ones1 = np.ones((1, N1s), f32)
    cselRe = np.concatenate([np.ones(N1), np.zeros(N1)])[None, :].astype(f32)
    cselIm = np.concatenate([np.zeros(N1), np.ones(N1)])[None, :].astype(f32)
    bv = np.asarray(b, f32).reshape(FIL, C)
    seedRe = (T * bv[:, 0::2]).reshape(1, FIL * CP)
    seedIm = (T * bv[:, 1::2]).reshape(1, FIL * CP)
    seeds = np.concatenate(
        [srow, ones1, cselRe, cselIm, seedRe, seedIm], axis=1)
    return {
        "xs": xs, "wr": wr, "wi": wi,
        "blob_r": cs["blob_r"],
        "blob_twb": cs["blob_twb"].astype(bf16),
        "blob_m2": cs["blob_m2"].astype(bf16),
        "blob_m3": cs["blob_m3"].astype(bf16),
        "blob_sel": cs["blob_sel"].astype(bf16),
        "seeds": seeds.astype(bf16),
        "cLAB": cs["cLAB"].astype(bf16),
    }


_NC_CACHE = {}


def kernel(x, w_real, w_imag, s, b):
    """Full-input entry point: shard over 8 cores, run, gather."""
    from concourse.bass_utils import run_bass_kernel_spmd
    cfg = FULL
    n_cores = 8
    if "full" not in _NC_CACHE:
        _NC_CACHE["full"] = build_nc(cfg)
    nc = _NC_CACHE["full"]
    x = np.asarray(x, dtype=np.float32)
    in_maps = [host_inputs(cfg, x[i * cfg.BL:(i + 1) * cfg.BL], w_real, w_imag, s, b)
               for i in range(n_cores)]
    res = run_bass_kernel_spmd(nc, in_maps, core_ids=list(range(n_cores)))
    outs = [np.asarray(res.results[i]["out"]).astype(np.float32)
            for i in range(n_cores)]
    return np.concatenate(outs, axis=0)


# revision 62
# speedup vs baseline: 1.0161x; 1.0161x over previous
"""Trainium2 Bass kernel v6: packed-complex Wiener deconvolution.

v6 over v5: even/odd channel pairs packed as complex rows (halves the
spectrum-multiply, M3 matmuls, T2 transposes, and evac traffic); filter
spectrum Hermitian-ized (g~ = (g[k]+conj(g[-k]))/2) via a P,Q dual-DFT with
sign-baked +/- M2 weight sets (no conj-flip indexing); |h|^2 pair-sum and the
+s regularizer folded into PE matmuls; bias enters through the M3 DC bin as a
rank-1 PE seed; M4 uses 3 cL components (Lr, -Li, +Li); the whole H->G chain
is quarter-pipelined across PE/ACT/DVE; Pool handles the (SBUF-only) x-twiddle
and part of the spectrum multiplies.
"""
import sys

sys.path.insert(0, "/opt/trn_rl_repo")

import numpy as np


def _get_cc():
    import concourse.bacc as bacc
    import concourse.mybir as mybir
    import concourse.tile as tile
    return bacc, mybir, tile


class Cfg:
    def __init__(self, T=8192, N2=128, N1=64, BL=2, C=8, FIL=16):
        assert N1 * N2 == T
        self.T, self.N2, self.N1, self.BL, self.C, self.FIL = T, N2, N1, BL, C, FIL
        self.CP = C // 2
        self.FC = FIL * C


FULL = Cfg()


def host_consts(cfg):
    T, N1, N2, FIL, CP = cfg.T, cfg.N1, cfg.N2, cfg.FIL, cfg.CP
    f32 = np.float32
    n2a, n1a, k2a, k1a = (np.arange(N2), np.arange(N1), np.arange(N2), np.arange(N1))
    cs = {}
    F2 = np.exp(-2j * np.pi * np.outer(n2a, k2a) / N2)          # [n2,k2]
    cs["blob_r"] = np.concatenate(
        [F2.real, F2.imag, -F2.imag], axis=1).astype(f32)
    Tw = np.exp(-2j * np.pi * np.outer(k2a, n1a) / T)           # [k2,n1]
    tw3 = np.concatenate([Tw.real, Tw.imag, -Tw.imag], axis=1).astype(f32)
    cs["blob_twb"] = tw3                                        # ->bf16 twiddles
    F1 = np.exp(-2j * np.pi * np.outer(n1a, k1a) / N1)          # [n1,k1]
    Wstd = np.vstack([np.hstack([F1.real, F1.imag]),
                      np.hstack([-F1.imag, F1.real])]).astype(f32)  # [n1s,k1s]
    Wre, Wim = Wstd[:, :N1], Wstd[:, N1:]
    # set+ out rows: [h+r; -h+i] = [Pr - Qi; -(Pi + Qr)]
    Wp_p = np.hstack([Wre, -Wim])
    Wq_p = np.hstack([-Wim, -Wre])
    # set- out rows: [h-r; h-i] = [Pr + Qi; -Pi + Qr]
    Wp_m = np.hstack([Wre, -Wim])
    Wq_m = np.hstack([Wim, Wre])
    cs["blob_m2"] = np.concatenate([Wstd, Wp_p, Wq_p, Wp_m, Wq_m], axis=1)
    F1b = np.exp(+2j * np.pi * np.outer(k1a, n1a) / N1)         # [k1,j]
    Fbr, Fbi = F1b.real, F1b.imag
    M3A = np.hstack([np.vstack([Fbr, -Fbi]), np.vstack([Fbi, Fbr])]) * 0.5
    M3B = np.hstack([np.vstack([-Fbi, -Fbr]), np.vstack([Fbr, -Fbi])]) * 0.5
    cs["blob_m3"] = np.concatenate([M3A, M3B], axis=1).astype(f32)
    I64 = np.eye(N1, dtype=f32)
    Spair = np.vstack([I64, I64])
    cs["blob_sel"] = np.hstack([Spair, Spair]).astype(f32)      # [k1s, 128]
    ia = np.arange(N2)
    L = np.exp(2j * np.pi * (np.outer(k2a, ia * N1)[:, None, :]
                             + k2a[:, None, None] * n1a[None, :, None]) / T) / T
    cs["cLAB"] = np.concatenate(
        [L.real.reshape(N2, N1 * N2), -L.imag.reshape(N2, N1 * N2),
         L.imag.reshape(N2, N1 * N2)],
        axis=1).astype(f32)                                     # [k2,(n1p,i)x3]
    return cs


def build_nc(cfg):
    bacc, mybir, tile = _get_cc()
    F32, F32R, BF16 = mybir.dt.float32, mybir.dt.float32r, mybir.dt.bfloat16
    AL = mybir.AluOpType
    T, N2, N1, BL, C, FIL, CP = (cfg.T, cfg.N2, cfg.N1, cfg.BL, cfg.C,
                                 cfg.FIL, cfg.CP)
    FC = cfg.FC
    N1s = 2 * N1                  # 128
    KF = FIL * N2                 # 2048
    HN = FIL * N1                 # 1024
    XN = BL * N1 * C              # 1024 (dram x layout, c innermost)
    CK = CP * N2                  # 512
    FH = FIL // 2                 # 8 filters per psum half-batch
    NF4 = FIL // 4                # 4 filters per G quarter
    QW = KF // 4                  # 512
    MCH = 512

    nc = bacc.Bacc("TRN2", debug=False)

    xs_d = nc.dram_tensor("xs", [N2, XN], F32R, kind="ExternalInput")
    wr_d = nc.dram_tensor("wr", [N2, HN], F32R, kind="ExternalInput")
    wi_d = nc.dram_tensor("wi", [N2, HN], F32R, kind="ExternalInput")
    blob_r_d = nc.dram_tensor("blob_r", [N2, 3 * N2], F32R, kind="ExternalInput")
    blob_twb_d = nc.dram_tensor("blob_twb", [N2, 3 * N1], BF16, kind="ExternalInput")
    blob_m2_d = nc.dram_tensor("blob_m2", [N1s, 5 * N1s], BF16, kind="ExternalInput")
    blob_m3_d = nc.dram_tensor("blob_m3", [N1s, 2 * N1s], BF16, kind="ExternalInput")
    blob_sel_d = nc.dram_tensor("blob_sel", [N1s, N1s], BF16, kind="ExternalInput")
    seeds_d = nc.dram_tensor("seeds", [1, KF + 3 * N1s + 2 * FIL * CP], BF16,
                             kind="ExternalInput")
    cLAB_d = nc.dram_tensor("cLAB", [N2, 3 * N1 * N2], BF16, kind="ExternalInput")
    out_d = nc.dram_tensor("out", [BL, T, FC], BF16, kind="ExternalOutput")

    with tile.TileContext(nc) as tc:
        from contextlib import ExitStack
        with tc.tile_pool(name="consts", bufs=1) as cpool, \
             tc.tile_pool(name="pers", bufs=1) as pers:
            # ---------- loads (SP queue, in order) ----------
            def load(name, shape, dt, dram):
                t = cpool.tile(shape, dt, tag=name, name=name)
                nc.sync.dma_start(out=t, in_=dram.ap())
                return t

            blob_r = load("blob_r", [N2, 3 * N2], F32R, blob_r_d)
            wtr = load("wtr", [N2, HN], F32R, wr_d)
            wti = load("wti", [N2, HN], F32R, wi_d)
            blob_twb = load("blob_twb", [N2, 3 * N1], BF16, blob_twb_d)
            blob_m2 = load("blob_m2", [N1s, 5 * N1s], BF16, blob_m2_d)
            blob_m3 = load("blob_m3", [N1s, 2 * N1s], BF16, blob_m3_d)
            blob_sel = load("blob_sel", [N1s, N1s], BF16, blob_sel_d)
            seeds = load("seeds", [1, KF + 3 * N1s + 2 * FIL * CP], BF16, seeds_d)
            xt = load("xt", [N2, XN], F32R, xs_d)
            cLAB = cpool.tile([N2, 3 * N1 * N2], BF16, tag="cLAB")
            QL = N1 * N2 // 4                                   # 2048

            F2r = blob_r[:, 0:N2]
            F2i = blob_r[:, N2:2 * N2]
            F2in = blob_r[:, 2 * N2:3 * N2]
            twrb = blob_twb[:, 0:N1]
            twib = blob_twb[:, N1:2 * N1]
            twinb = blob_twb[:, 2 * N1:3 * N1]
            cM2x = blob_m2[:, 0:N1s]
            Wp_p = blob_m2[:, N1s:2 * N1s]
            Wq_p = blob_m2[:, 2 * N1s:3 * N1s]
            Wp_m = blob_m2[:, 3 * N1s:4 * N1s]
            Wq_m = blob_m2[:, 4 * N1s:5 * N1s]
            cM3A = blob_m3[:, 0:N1s]
            cM3B = blob_m3[:, N1s:2 * N1s]
            selA = blob_sel[:, 0:N1s]
            srow = seeds[:, 0:KF]
            ones1 = seeds[:, KF:KF + N1s]
            cselRe = seeds[:, KF + N1s:KF + 2 * N1s]
            cselIm = seeds[:, KF + 2 * N1s:KF + 3 * N1s]
            seedRe = seeds[:, KF + 3 * N1s:KF + 3 * N1s + FIL * CP]
            seedIm = seeds[:, KF + 3 * N1s + FIL * CP:]
            cLA = cLAB[:, 0:N1 * N2]
            cLB = cLAB[:, N1 * N2:2 * N1 * N2]
            cLC = cLAB[:, 2 * N1 * N2:3 * N1 * N2]

            # persistent tiles
            Z0A = pers.tile([N1s, BL * CK], BF16, tag="Z0A")     # [k1s,(b,cp,k2)]
            Grep = pers.tile([N1s, FIL * 2 * N2], BF16, tag="Grep")
            Grv = Grep.rearrange("p (f m q) -> p f m q", f=FIL, m=2)
            hpm = pers.tile([N1s, KF], BF16, tag="hpm")          # [h+r; -h+i]
            hmm = pers.tile([N1s, KF], BF16, tag="hmm")          # [h-r; h-i]
            sqp = pers.tile([N1s, KF], BF16, tag="sqp")
            sqm = pers.tile([N1s, KF], BF16, tag="sqm")
            RP = pers.tile([N1s, KF], BF16, tag="RP")            # r+ both halves
            RM = pers.tile([N1s, KF], BF16, tag="RM")            # r- both halves
            G12s = pers.tile([N1s, QW], BF16, tag="G12s")
            SS2p = pers.tile([N1s, QW], BF16, tag="SS2p")
            SS2m = pers.tile([N1s, QW], BF16, tag="SS2m")

            fes = ExitStack()
            fwd = fes.enter_context(tc.tile_pool(name="fwd", bufs=1))
            pxes = ExitStack()
            pAx = pxes.enter_context(tc.tile_pool(name="pAx", bufs=1, space="PSUM"))
            phes = ExitStack()
            pAh = phes.enter_context(tc.tile_pool(name="pAh", bufs=1, space="PSUM"))

            # ---------- M1: H quarter-pipelined, x interleaved ----------
            SH = fwd.tile([N2, 2 * 2 * FIL * N1], BF16, tag="SH")
            SHv = SH.rearrange("p (g m f n) -> p g m f n", g=2, m=2, f=FIL)
            SX = fwd.tile([N2, 2 * BL * CP * N1], BF16, tag="SX")
            SXv = SX.rearrange("p (m b c n) -> p m b c n", m=2, b=BL, c=CP)
            wtrv = wtr.rearrange("p (f n) -> p f n", f=FIL)
            wtiv = wti.rearrange("p (f n) -> p f n", f=FIL)
            Ax = pAx.tile([N2, 2 * BL * CP * N1], F32, tag="Ax")  # [k2,(m,b,cp,n1)]
            Axv = Ax.rearrange("p (m b c n) -> p m b c n", m=2, b=BL, c=CP)
            xtv = xt.rearrange("p (b n c e) -> p b n c e", b=BL, n=N1, c=CP)
            xe = xtv[:, :, :, :, 0].transpose([0, 1, 3, 2])      # [n2,(b,cp,n1)]
            xo = xtv[:, :, :, :, 1].transpose([0, 1, 3, 2])

            def m1h_q(q):
                Ahh = pAh.tile([N2, 2 * 2 * NF4 * N1], F32, tag="Ah", name=f"Ah{q}")
                Av = Ahh.rearrange("p (g m f n) -> p g m f n", g=2, m=2, f=NF4)
                fsl = slice(q * NF4, (q + 1) * NF4)
                nc.tensor.matmul(Av[:, 0, 0], F2r, wtrv[:, fsl], start=True, stop=True)
                nc.tensor.matmul(Av[:, 0, 1], F2i, wtrv[:, fsl], start=True, stop=True)
                nc.tensor.matmul(Av[:, 1, 0], F2r, wtiv[:, fsl], start=True, stop=True)
                nc.tensor.matmul(Av[:, 1, 1], F2i, wtiv[:, fsl], start=True, stop=True)
                nc.scalar.copy(out=SHv[:, 0, :, fsl, :], in_=Av[:, 0])
                nc.scalar.copy(out=SHv[:, 1, :, fsl, :], in_=Av[:, 1])

            m1h_q(0)
            m1h_q(1)
            nc.tensor.matmul(Axv[:, 0], F2r, xe, start=True, stop=False)
            nc.tensor.matmul(Axv[:, 0], F2in, xo, start=False, stop=True)
            nc.tensor.matmul(Axv[:, 1], F2i, xe, start=True, stop=False)
            nc.tensor.matmul(Axv[:, 1], F2r, xo, start=False, stop=True)
            nc.scalar.copy(out=SX, in_=Ax)
            m1h_q(2)
            m1h_q(3)

            # ---------- H twiddle (DVE, bf16 2x), per quarter ----------
            Bh = fwd.tile([N2, 2 * FIL * 2 * N1], BF16, tag="Bh")
            Bhv = Bh.rearrange("p (u g f m n) -> p u g f m n", u=4, g=2, f=NF4, m=2)
            uh = fwd.tile([N2, NF4 * N1], BF16, tag="uh")
            vh = fwd.tile([N2, NF4 * N1], BF16, tag="vh")
            uhv = uh.rearrange("p (f n) -> p f n", f=NF4)
            vhv = vh.rearrange("p (f n) -> p f n", f=NF4)

            def bch(w):
                return w[:, None, :].broadcast_to([N2, NF4, N1])

            BTH = fwd.tile([N1s, 2 * 2 * FH * N2], BF16, tag="BTH")
            BTHg = BTH.rearrange("p (u g f q) -> p u g f q", u=4, g=2, f=NF4)

            uh2 = fwd.tile([N2, NF4 * N1], BF16, tag="uh2")
            vh2 = fwd.tile([N2, NF4 * N1], BF16, tag="vh2")
            uh2v = uh2.rearrange("p (f n) -> p f n", f=NF4)
            vh2v = vh2.rearrange("p (f n) -> p f n", f=NF4)

            def htw_q(u, eng=None, us=None, vs=None):
                eng = eng or nc.vector
                us, vs = us or uhv, vs or vhv
                fsl = slice(u * NF4, (u + 1) * NF4)
                for g in range(2):    # P, Q
                    eng.tensor_tensor(out=us, in0=SHv[:, g, 0, fsl, :],
                                      in1=bch(twrb), op=AL.mult)
                    eng.tensor_tensor(out=vs, in0=SHv[:, g, 1, fsl, :],
                                      in1=bch(twinb), op=AL.mult)
                    eng.tensor_tensor(out=Bhv[:, u, g, :, 0, :], in0=us,
                                      in1=vs, op=AL.mult if False else AL.add)
                    eng.tensor_tensor(out=us, in0=SHv[:, g, 0, fsl, :],
                                      in1=bch(twib), op=AL.mult)
                    eng.tensor_tensor(out=vs, in0=SHv[:, g, 1, fsl, :],
                                      in1=bch(twrb), op=AL.mult)
                    eng.tensor_tensor(out=Bhv[:, u, g, :, 1, :], in0=us,
                                      in1=vs, op=AL.add)

            def t1h_q(u):
                nc.sync.dma_start_transpose(
                    out=BTHg[:, u].rearrange("p g f q -> p (g f) q"),
                    in_=Bhv[:, u].rearrange("p g f m n -> p (g f) (m n)"))

            htw_q(0)
            htw_q(1)
            htw_q(3)

            # ---------- x twiddle (Pool, staged bf16 SBUF), split per b ----------
            Bc = fwd.tile([N2, BL * CP * 2 * N1], BF16, tag="Bc")
            Bcv = Bc.rearrange("p (b c m n) -> p b c m n", b=BL, c=CP, m=2)
            ux = fwd.tile([N2, CP * N1], BF16, tag="ux")
            vx = fwd.tile([N2, CP * N1], BF16, tag="vx")
            uxv = ux.rearrange("p (c n) -> p c n", c=CP)
            vxv = vx.rearrange("p (c n) -> p c n", c=CP)

            def bcx(w):
                return w[:, None, :].broadcast_to([N2, CP, N1])

            def xtw_b(b):
                nc.gpsimd.tensor_tensor(out=uxv, in0=SXv[:, 0, b], in1=bcx(twrb),
                                        op=AL.mult)
                nc.gpsimd.tensor_tensor(out=vxv, in0=SXv[:, 1, b], in1=bcx(twinb),
                                        op=AL.mult)
                nc.gpsimd.tensor_tensor(out=Bcv[:, b, :, 0, :], in0=uxv, in1=vxv,
                                        op=AL.add)
                nc.gpsimd.tensor_tensor(out=uxv, in0=SXv[:, 0, b], in1=bcx(twib),
                                        op=AL.mult)
                nc.gpsimd.tensor_tensor(out=vxv, in0=SXv[:, 1, b], in1=bcx(twrb),
                                        op=AL.mult)
                nc.gpsimd.tensor_tensor(out=Bcv[:, b, :, 1, :], in0=uxv, in1=vxv,
                                        op=AL.add)

            xtw_b(1)
            htw_q(2, eng=nc.gpsimd, us=uh2v, vs=vh2v)
            xtw_b(0)

            # ---------- T1s (SP, ordered by expected readiness) ----------
            BTx = fwd.tile([N1s, BL * CP * N2], BF16, tag="BTx")   # [n1s,(b,cp,k2)]
            BTxv = BTx.rearrange("p (b c q) -> p b c q", b=BL, c=CP)

            def t1x_b(b):
                nc.sync.dma_start_transpose(
                    out=BTxv[:, b].rearrange("p c q -> p c q"),
                    in_=Bcv[:, b].rearrange("p c m n -> p c (m n)"))

            t1h_q(0)
            t1x_b(1)
            t1h_q(1)
            t1h_q(3)
            t1h_q(2)
            t1x_b(0)
            for qc in range(12):
                nc.sync.dma_start(out=cLAB[:, qc * QL:(qc + 1) * QL],
                                  in_=cLAB_d.ap()[:, qc * QL:(qc + 1) * QL])

            # ---------- quarter-pipelined M2h/squares/SS/recip + M2x ----------
            phes.close()
            pxes.close()
            pZes = ExitStack()
            pZ = pZes.enter_context(tc.tile_pool(name="pZ", bufs=1, space="PSUM"))
            pHes = ExitStack()
            pH = pHes.enter_context(tc.tile_pool(name="pH", bufs=3, space="PSUM"))
            pSes = ExitStack()
            pS = pSes.enter_context(tc.tile_pool(name="pS", bufs=2, space="PSUM"))

            def m2h_q(q):
                """Quarter q of both +/- sets -> hpm/hmm + squares."""
                qs = slice(q * QW, (q + 1) * QW)
                for (Wp_, Wq_, dsth, dstsq, nm) in (
                        (Wp_p, Wq_p, hpm, sqp, "p"), (Wp_m, Wq_m, hmm, sqm, "m")):
                    Hq = pH.tile([N1s, QW], F32, tag="Hq", name=f"Hq{nm}{q}")
                    nc.tensor.matmul(Hq, Wp_, BTHg[:, q, 0].rearrange(
                        "p f q -> p (f q)"), start=True, stop=False)
                    nc.tensor.matmul(Hq, Wq_, BTHg[:, q, 1].rearrange(
                        "p f q -> p (f q)"), start=False, stop=True)
                    nc.scalar.square(dstsq[:, qs], Hq)
                    nc.scalar.copy(out=dsth[:, qs], in_=Hq)

            def ss_q(q, stage=False):
                qs = slice(q * QW, (q + 1) * QW)
                for (sqt, rrt, st) in ((sqp, RP, SS2p), (sqm, RM, SS2m)):
                    nm = "p" if sqt is sqp else "m"
                    SSq = pS.tile([N1s, QW], F32, tag="SSq", name=f"SS{nm}{q}")
                    nc.tensor.matmul(SSq, selA, sqt[:, qs], start=True, stop=False)
                    nc.tensor.matmul(SSq, ones1, srow[:, qs], start=False, stop=True)
                    if stage:
                        nc.scalar.copy(out=st, in_=SSq)
                    else:
                        with nc.allow_low_precision(reason="bf16 wiener gain"):
                            nc.vector.reciprocal(out=rrt[:, qs], in_=SSq)

            def recip_q2():
                qs = slice(2 * QW, 3 * QW)
                with nc.allow_low_precision(reason="bf16 wiener gain"):
                    nc.vector.reciprocal(out=RP[:, qs], in_=SS2p)
                    nc.vector.reciprocal(out=RM[:, qs], in_=SS2m)

            def g_quarter(q):
                # G12 rows: [G1(k1); G2(k1)] = hpm*RP + hmm*RM (all aligned)
                qs = slice(q * QW, (q + 1) * QW)
                fq = slice(q * NF4, (q + 1) * NF4)
                nc.vector.tensor_tensor(out=G12s, in0=hpm[:, qs], in1=RP[:, qs],
                                        op=AL.mult)
                hmv = hmm.rearrange("p (f q) -> p f q", f=FIL)[:, fq, :]
                rmv = RM.rearrange("p (f q) -> p f q", f=FIL)[:, fq, :]
                nc.vector.tensor_tensor(out=Grv[:, fq, 0, :], in0=hmv, in1=rmv,
                                        op=AL.mult)
                g12v = G12s.rearrange("p (f q) -> p f q", f=NF4)
                nc.vector.tensor_tensor(out=Grv[:, fq, 0, :],
                                        in0=Grv[:, fq, 0, :], in1=g12v, op=AL.add)
                # rows now [G1; G2] in slot m=0; scatter to (m, halves)
                nc.vector.tensor_copy(out=Grv[:N1, fq, 1, :], in_=Grv[N1:, fq, 0, :])
                nc.vector.tensor_copy(out=Grv[N1:, fq, 1, :], in_=Grv[N1:, fq, 0, :])
                nc.vector.tensor_copy(out=Grv[N1:, fq, 0, :], in_=Grv[:N1, fq, 0, :])

            # software-pipelined emission: PE one stage ahead of evac deps
            Zps = pZ.tile([N1s, BL * CK], F32, tag="Zps")

            def m2x_b(b):
                bsl = slice(b * CK, (b + 1) * CK)
                nc.tensor.matmul(Zps[:, bsl], cM2x, BTx[:, bsl], start=True,
                                 stop=True)
                nc.scalar.copy(out=Z0A[:, bsl], in_=Zps[:, bsl])

            m2h_q(0)
            m2h_q(1)
            ss_q(0)
            m2x_b(1)
            m2h_q(3)
            ss_q(1)
            ss_q(3)
            g_quarter(0)
            m2h_q(2)
            m2x_b(0)
            ss_q(2)
            pSes.close()
            pHes.close()
            pZes.close()
            fes.close()

            # ================= inverse =================
            zvA = Z0A.rearrange("p (b c q) -> p b c q", b=BL, c=CP)
            with tc.tile_pool(name="dt", bufs=1) as dtp, \
                 tc.tile_pool(name="stg", bufs=1) as stp, \
                 tc.tile_pool(name="zt", bufs=4) as ztp, \
                 tc.tile_pool(name="cse", bufs=5) as csp, \
                 tc.tile_pool(name="ddp", bufs=2, space="PSUM") as ddp, \
                 tc.tile_pool(name="yp", bufs=2, space="PSUM") as yps:
                DT0 = dtp.tile([N2, FIL * CP * N1s], BF16, tag="DT0")
                DT1 = dtp.tile([N2, FIL * CP * N1s], BF16, tag="DT1")
                DT = [DT0, DT1]                       # [k2,(f,cp,m,n1')]
                STG0 = stp.tile([N2, N1 * FC], BF16, tag="STG0")
                STG1 = stp.tile([N2, N1 * FC], BF16, tag="STG1")
                STG = [STG0, STG1]                    # [i,(j,f,cp,m)]
                def new_cseq():
                    return csp.tile([N1s, 4 * CK], BF16, tag="cseq", name="cseq")
                srv = seedRe.rearrange("o (f c) -> o f c", f=FIL)
                siv = seedIm.rearrange("o (f c) -> o f c", f=FIL)

                # Pool zt pairs are prefetched one loop-step early
                POOL_ZT = {(1, 2), (1, 5), (0, 2), (0, 5)}

                def zt_mul(b, fp, eng):
                    f0 = 2 * fp
                    zt = ztp.tile([N1s, 2 * 2 * CK], BF16, tag="zt")
                    ztv = zt.rearrange("p (i m c q) -> p i m c q", i=2, m=2, c=CP)
                    g12 = Grv[:, f0:f0 + 2][:, :, :, None, :].broadcast_to(
                        [N1s, 2, 2, CP, N2])
                    zin = zvA[:, b][:, None, None, :, :].broadcast_to(
                        [N1s, 2, 2, CP, N2])
                    eng.tensor_tensor(out=ztv, in0=zin, in1=g12, op=AL.mult)
                    return zt

                PENDING_ZT = {}

                def cmul_m3_pair(b, fp, cseq):
                    """f = 2*fp, 2*fp+1: 8+4 matmuls, one evac (ACT)."""
                    f0 = 2 * fp
                    zt = PENDING_ZT.pop((b, fp), None)
                    if zt is None:
                        zt = zt_mul(b, fp, nc.vector)
                    ztv = zt.rearrange("p (i m c q) -> p i m c q", i=2, m=2, c=CP)
                    DD = ddp.tile([N1s, 2 * CK], F32, tag="DD")
                    for i in range(2):
                        f = f0 + i
                        sl = DD[:, i * CK:(i + 1) * CK]
                        nc.tensor.matmul(sl, cM3A, ztv[:, i, 0].rearrange(
                            "p c q -> p (c q)"), start=True, stop=False)
                        nc.tensor.matmul(sl, cM3B, ztv[:, i, 1].rearrange(
                            "p c q -> p (c q)"), start=False, stop=False)
                        DDv = sl.rearrange("p (c q) -> p c q", c=CP)
                        nc.tensor.matmul(DDv[:, :, 0:1], cselRe,
                                         srv[:, f, :, None], start=False, stop=False)
                        nc.tensor.matmul(DDv[:, :, 0:1], cselIm,
                                         siv[:, f, :, None], start=False, stop=True)
                    dst = cseq[:, (fp % 2) * 2 * CK:(fp % 2 + 1) * 2 * CK]
                    nc.scalar.copy(out=dst, in_=DD)

                def prefetch_pool_zt(b, fp):
                    if (b, fp) in POOL_ZT:
                        PENDING_ZT[(b, fp)] = zt_mul(b, fp, nc.gpsimd)

                def quad_t2(b, qf, cseq):
                    dtv = DT[b].rearrange("p (f c n) -> p (f c) n", f=FIL, c=CP)
                    nc.sync.dma_start_transpose(
                        out=dtv[:, qf * 16:(qf + 1) * 16, :], in_=cseq)

                def pair_t2(b, fp, cseq):
                    dtv = DT[b].rearrange("p (f c n) -> p (f c) n", f=FIL, c=CP)
                    sl = cseq[:, (fp % 2) * 2 * CK:(fp % 2 + 1) * 2 * CK]
                    nc.sync.dma_start_transpose(
                        out=dtv[:, fp * 8:(fp + 1) * 8, :], in_=sl)

                def m4_group8(b, g0, eng="act"):
                    """8 n1p values; ypsum [i,(j8,m,fc64)]; one evac."""
                    dtm = DT[b].rearrange("p (f c m n) -> p n m f c",
                                          f=FIL, c=CP, m=2)
                    ypsum = yps.tile([N2, 8 * 2 * N1], F32, tag="yps")
                    ypv = ypsum.rearrange("p (j m o) -> p j m o", j=8, m=2)
                    for j in range(8):
                        n1p = g0 + j
                        wA = cLA[:, n1p * N2:(n1p + 1) * N2]
                        wB = cLB[:, n1p * N2:(n1p + 1) * N2]
                        wC = cLC[:, n1p * N2:(n1p + 1) * N2]
                        dr = dtm[:, n1p, 0]
                        di = dtm[:, n1p, 1]
                        nc.tensor.matmul(ypv[:, j, 0], wA, dr, start=True, stop=False)
                        nc.tensor.matmul(ypv[:, j, 0], wB, di, start=False, stop=True)
                        nc.tensor.matmul(ypv[:, j, 1], wA, di, start=True, stop=False)
                        nc.tensor.matmul(ypv[:, j, 1], wC, dr, start=False, stop=True)
                    dst = STG[b].rearrange("p (n f c m) -> p n f c m",
                                           n=N1, f=FIL, c=CP)[:, g0:g0 + 8]
                    src = ypv.rearrange("p j m (f c) -> p j f c m", f=FIL)
                    if eng == "act":
                        nc.scalar.copy(out=dst, in_=src)
                    else:
                        nc.vector.tensor_copy(out=dst, in_=src)

                def m4_group8_h(b, g0, fh, eng="act"):
                    """8 n1p values, HALF the filters (fh*8..fh*8+8)."""
                    dtm = DT[b].rearrange("p (f c m n) -> p n m f c",
                                          f=FIL, c=CP, m=2)
                    fsl = slice(fh * FH, (fh + 1) * FH)
                    ypsum = yps.tile([N2, 8 * 2 * N1], F32, tag="yps")
                    ypv = ypsum.rearrange("p (j m o) -> p j m o", j=8, m=2)
                    hw2 = FH * CP
                    for j in range(8):
                        n1p = g0 + j
                        wA = cLA[:, n1p * N2:(n1p + 1) * N2]
                        wB = cLB[:, n1p * N2:(n1p + 1) * N2]
                        wC = cLC[:, n1p * N2:(n1p + 1) * N2]
                        dr = dtm[:, n1p, 0, fsl, :]
                        di = dtm[:, n1p, 1, fsl, :]
                        o0 = ypv[:, j, 0, :hw2]
                        o1 = ypv[:, j, 1, :hw2]
                        nc.tensor.matmul(o0, wA, dr, start=True, stop=False)
                        nc.tensor.matmul(o0, wB, di, start=False, stop=True)
                        nc.tensor.matmul(o1, wA, di, start=True, stop=False)
                        nc.tensor.matmul(o1, wC, dr, start=False, stop=True)
                    dst = STG[b].rearrange("p (n f c m) -> p n f c m",
                                           n=N1, f=FIL, c=CP)[:, g0:g0 + 8, fsl]
                    src_ = ypv[:, :, :, :hw2].rearrange(
                        "p j m (f c) -> p j f c m", f=FH)
                    if eng == "act":
                        nc.scalar.copy(out=dst, in_=src_)
                    else:
                        nc.vector.tensor_copy(out=dst, in_=src_)

                def out_chunk(b, g0, gn=16):
                    nc.scalar.dma_start(
                        out=out_d.ap()[b].rearrange(
                            "(q n) fc -> q (n fc)", n=N1)[:, g0 * FC:(g0 + gn) * FC],
                        in_=STG[b][:, g0 * FC:(g0 + gn) * FC])

                EV = ["dve", "act"]
                # ----- loop 1: b=1 M3 (8 pair-steps) -----
                cseq = new_cseq()
                for fp in range(FIL // 2):
                    if fp in (1, 3, 5):
                        g_quarter((fp + 1) // 2)
                    prefetch_pool_zt(1, fp + 1)
                    cmul_m3_pair(1, fp, cseq)
                    if fp % 2 == 1:
                        quad_t2(1, fp // 2, cseq)
                        cseq = new_cseq()
                prefetch_pool_zt(0, 0)
                # ----- loop 2: b=0 M3 + all 8 b=1 M4 super-groups -----
                for fp in range(FIL // 2):
                    if fp < 6:
                        m4_group8(1, fp * 8, EV[fp % 2])
                        if fp % 2 == 1 and fp >= 3:
                            out_chunk(1, (fp - 3) * 8)
                        prefetch_pool_zt(0, fp + 1)
                        cmul_m3_pair(0, fp, cseq)
                    else:
                        prefetch_pool_zt(0, fp + 1)
                        cmul_m3_pair(0, fp, cseq)
                        m4_group8(1, fp * 8, EV[fp % 2])
                    if fp >= 6:
                        pair_t2(0, fp, cseq)
                        if fp % 2 == 1:
                            cseq = new_cseq()
                    elif fp % 2 == 1:
                        quad_t2(0, fp // 2, cseq)
                        cseq = new_cseq()
                out_chunk(1, 32)
                out_chunk(1, 48)
                # PE warm-fill: keep clock at full speed across the DT0 wait
                fillDD = ddp.tile([N1s, 2 * CK], F32, tag="DD", name="fillDD")
                for wf in range(18):
                    nc.tensor.matmul(fillDD[:, :CK], cM3A, Grep[:, :CK],
                                     start=True, stop=True)
                # ----- b=0 M4 tail -----
                for g in range(8):
                    m4_group8(0, g * 8, EV[g % 2])
                    if g % 2 == 1 and g < 7:
                        out_chunk(0, g * 8 - 8)
                out_chunk(0, 48, gn=12)
                out_chunk(0, 60, gn=4)

    nc.compile()
    return nc


def host_inputs(cfg, x_sh, w_real, w_imag, s, b):
    """Per-core in_map (numpy) for one core's batch shard. Layout-only on
    inputs; constants precomputed."""
    import ml_dtypes
    cs = host_consts(cfg)
    T, N1, N2, FIL, C, CP, BL = (cfg.T, cfg.N1, cfg.N2, cfg.FIL, cfg.C,
                                 cfg.CP, cfg.BL)
    FC, KF, N1s = cfg.FC, FIL * N2, 2 * N1
    f32, bf16 = np.float32, ml_dtypes.bfloat16
    x_sh = np.asarray(x_sh, f32)
    xs = np.ascontiguousarray(
        x_sh.reshape(BL, N2, N1, C).transpose(1, 0, 2, 3)).reshape(N2, BL * N1 * C)
    wr = np.ascontiguousarray(
        np.asarray(w_real, f32).reshape(FIL, N2, N1).transpose(1, 0, 2)
    ).reshape(N2, FIL * N1)
    wi = np.ascontiguousarray(
        np.asarray(w_imag, f32).reshape(FIL, N2, N1).transpose(1, 0, 2)
    ).reshape(N2, FIL * N1)
    sv = np.asarray(s, f32).reshape(FIL)
    srow = np.repeat(sv, N2)[None, :]                       # [1,(f,k2)]
    ones1 = np.ones((1, N1s), f32)
    cselRe = np.concatenate([np.ones(N1), np.zeros(N1)])[None, :].astype(f32)
    cselIm = np.concatenate([np.zeros(N1), np.ones(N1)])[None, :].astype(f32)
    bv = np.asarray(b, f32).reshape(FIL, C)
    seedRe = (T * bv[:, 0::2]).reshape(1, FIL * CP)
    seedIm = (T * bv[:, 1::2]).reshape(1, FIL * CP)
    seeds = np.concatenate(
        [srow, ones1, cselRe, cselIm, seedRe, seedIm], axis=1)
    return {
        "xs": xs, "wr": wr, "wi": wi,
        "blob_r": cs["blob_r"],
        "blob_twb": cs["blob_twb"].astype(bf16),
        "blob_m2": cs["blob_m2"].astype(bf16),
        "blob_m3": cs["blob_m3"].astype(bf16),
        "blob_sel": cs["blob_sel"].astype(bf16),
        "seeds": seeds.astype(bf16),
        "cLAB": cs["cLAB"].astype(bf16),
    }


_NC_CACHE = {}


def kernel(x, w_real, w_imag, s, b):
    """Full-input entry point: shard over 8 cores, run, gather."""
    from concourse.bass_utils import run_bass_kernel_spmd
    cfg = FULL
    n_cores = 8
    if "full" not in _NC_CACHE:
        _NC_CACHE["full"] = build_nc(cfg)
    nc = _NC_CACHE["full"]
    x = np.asarray(x, dtype=np.float32)
    in_maps = [host_inputs(cfg, x[i * cfg.BL:(i + 1) * cfg.BL], w_real, w_imag, s, b)
               for i in range(n_cores)]
    res = run_bass_kernel_spmd(nc, in_maps, core_ids=list(range(n_cores)))
    outs = [np.asarray(res.results[i]["out"]).astype(np.float32)
            for i in range(n_cores)]
    return np.concatenate(outs, axis=0)
